# revision 1
# baseline (speedup 1.0000x reference)
"""BitLinear (BitNet 1.58 absmean ternary) forward on 8 trn2 NeuronCores.

Math:  gamma = mean(|W|) + 1e-8
       Wq    = clip(round(W/gamma), -1, 1)   ==  sign(w) * [|w| > gamma/2]
       out   = x @ Wq^T + bias

Sharding: data-parallel over x rows (B*S = 16384 -> 2048 rows/core),
W replicated, so gamma is computed redundantly per core (no collective).

Per-core device kernel:
  - gamma: DVE abs-reduce over a bf16 copy of W, cross-partition sum via a
    ones-matmul on PE (bf16 rounding of |W| perturbs the mean by ~2e-7
    relative, same order as fp32 summation-order noise).
  - quantize on the fly from fp32 W^T stream:
      2*Wq = Sign(w - gamma/2) + Sign(w + gamma/2)  in {-2, 0, 2}, exact bf16
    and x is scaled by 0.5 during its bf16 cast to compensate exactly.
  - out^T[o, r] = sum_i (2Wq)^T[i,o] . (x/2)^T[i,r] : bf16 matmuls, N=512,
    fp32 PSUM accumulation, bias added during the PSUM->SBUF copy.
"""

import os
import sys

for _p in (
    "/root/.axon_site",
    "/root/.axon_site/_ro/trn_rl_repo",
    "/root/.axon_site/_ro/pypackages",
    "/opt/trn_rl_repo",
):
    if os.path.isdir(_p) and _p not in sys.path:
        sys.path.append(_p)

import numpy as np
import ml_dtypes

import concourse.bass as bass
import concourse.tile as tile
from concourse import bacc, mybir
from concourse.bass import ts
from concourse.bass_utils import run_bass_kernel_spmd

AF = mybir.ActivationFunctionType
F32 = mybir.dt.float32
BF16 = mybir.dt.bfloat16

N_CORES = 8
P = 128
RC = 512  # matmul moving free dim / psum bank


def build_bitlinear_program(R, D, O, n_cores=N_CORES):
    """Build the per-core SPMD program.

    DRAM inputs (per core):
      xT   [D, R]           fp32   x shard, transposed (i, r), pre-scaled? no - raw
      wts  [O//128, 128, D] fp32   W^T swizzled: wts[ob, ki, kb*128+oi] = W[ob*128+oi, kb*128+ki]
      wg   [128, D*O//128]  bf16   W cast to bf16 (any layout), gamma source
      biasv [O]             fp32
    DRAM output:
      outT [O, R]           fp32   out^T shard (o, r)
    """
    assert R % RC == 0 and D % P == 0 and O % P == 0
    n_rc = R // RC
    n_kb = D // P
    n_ob = O // P
    WCH = min(1024, D)  # fp32 W chunk for quantization
    n_wch = D // WCH
    G_FREE = (D * O) // P
    GT = min(4096, G_FREE)  # gamma tile free size
    n_gt = G_FREE // GT
    assert G_FREE % GT == 0

    nc = bacc.Bacc(
        "TRN2",
        target_bir_lowering=False,
        debug=False,
        num_devices=n_cores,
    )
    xT = nc.dram_tensor("xT", [D, R], F32, kind="ExternalInput").ap()
    wts = nc.dram_tensor("wts", [n_ob, P, D], F32, kind="ExternalInput").ap()
    wg = nc.dram_tensor("wg", [P, G_FREE], BF16, kind="ExternalInput").ap()
    biasv = nc.dram_tensor("biasv", [O], F32, kind="ExternalInput").ap()
    outT = nc.dram_tensor("outT", [O, R], F32, kind="ExternalOutput").ap()

    with tile.TileContext(nc) as tc:
        with (
            tc.tile_pool(name="small", bufs=1) as small,
            tc.tile_pool(name="gpool", bufs=2) as gpool,
            tc.tile_pool(name="xf", bufs=2) as xf_pool,
            tc.tile_pool(name="xb", bufs=1) as xb_pool,
            tc.tile_pool(name="wf", bufs=3) as wf_pool,
            tc.tile_pool(name="sgn", bufs=2) as sgn_pool,
            tc.tile_pool(name="wq", bufs=2) as wq_pool,
            tc.tile_pool(name="osb", bufs=4) as osb_pool,
            tc.tile_pool(name="ps", bufs=4, space="PSUM") as ps_pool,
            tc.tile_pool(name="psg", bufs=1, space="PSUM") as psg_pool,
        ):
            # ---- constants / bias ----
            ones = small.tile([P, 1], F32)
            nc.vector.memset(ones[:], 1.0)
            bias_sb = small.tile([P, n_ob], F32)
            with nc.allow_non_contiguous_dma(reason="tiny one-shot bias load"):
                nc.sync.dma_start(
                    bias_sb[:], biasv.rearrange("(ob oi) -> oi ob", oi=P)
                )

            # ---- gamma = mean(|W|) + 1e-8 (threshold gamma/2) ----
            pacc = small.tile([P, n_gt], F32)
            for t in range(n_gt):
                g = gpool.tile([P, GT], BF16)
                nc.sync.dma_start(g[:], wg[:, ts(t, GT)])
                nc.vector.tensor_reduce(
                    out=pacc[:, t : t + 1],
                    in_=g[:],
                    axis=mybir.AxisListType.X,
                    op=mybir.AluOpType.add,
                    apply_absolute_value=True,
                )
            pacc1 = small.tile([P, 1], F32)
            nc.vector.reduce_sum(pacc1[:], pacc[:], axis=mybir.AxisListType.X)
            ps_g = psg_pool.tile([1, 1], F32)
            nc.tensor.matmul(ps_g[:], pacc1[:], ones[:], start=True, stop=True)
            halfg = small.tile([1, 1], F32)
            # gamma/2 = sum/(D*O) * 0.5 + 0.5e-8
            nc.vector.tensor_scalar(
                halfg[:],
                ps_g[:],
                0.5 / float(D * O),
                0.5e-8,
                mybir.AluOpType.mult,
                mybir.AluOpType.add,
            )
            neghalfg = small.tile([1, 1], F32)
            nc.vector.tensor_scalar_mul(neghalfg[:], halfg[:], -1.0)
            halfg_b = small.tile([P, 1], F32)
            neghalfg_b = small.tile([P, 1], F32)
            nc.gpsimd.partition_broadcast(halfg_b[:], halfg[:])
            nc.gpsimd.partition_broadcast(neghalfg_b[:], neghalfg[:])

            # ---- on-the-fly ternary quantization of one W^T block ----
            def quantize_ob(ob):
                wq2 = wq_pool.tile([P, D], BF16)
                for ch in range(n_wch):
                    wf = wf_pool.tile([P, WCH], F32)
                    nc.sync.dma_start(wf[:], wts[ob, :, ts(ch, WCH)])
                    s1 = sgn_pool.tile([P, WCH], BF16, tag="s1")
                    s2 = sgn_pool.tile([P, WCH], BF16, tag="s2")
                    nc.scalar.activation(s1[:], wf[:], AF.Sign, bias=neghalfg_b[:, 0:1])
                    nc.scalar.activation(s2[:], wf[:], AF.Sign, bias=halfg_b[:, 0:1])
                    nc.vector.tensor_add(
                        out=wq2[:, ts(ch, WCH)], in0=s1[:], in1=s2[:]
                    )
                return wq2

            # quantize first block before x phase so ACT starts on it early
            wq2_first = quantize_ob(0)

            # ---- x load + bf16 cast with 0.5 scale folded in ----
            xbf = xb_pool.tile([P, n_kb, R], BF16)
            for kb in range(n_kb):
                xf = xf_pool.tile([P, R], F32)
                nc.sync.dma_start(xf[:], xT[ts(kb, P), :])
                nc.scalar.activation(
                    xbf[:, kb, :], xf[:], AF.Copy, bias=0.0, scale=0.5
                )

            # ---- main: out^T[ob, rc] = sum_kb wq2^T . xbf ----
            for ob in range(n_ob):
                wq2 = wq2_first if ob == 0 else quantize_ob(ob)
                for rc in range(n_rc):
                    ps = ps_pool.tile([P, RC], F32)
                    for kb in range(n_kb):
                        nc.tensor.matmul(
                            ps[:],
                            wq2[:, ts(kb, P)],
                            xbf[:, kb, ts(rc, RC)],
                            start=(kb == 0),
                            stop=(kb == n_kb - 1),
                        )
                    osb = osb_pool.tile([P, RC], F32)
                    nc.scalar.activation(
                        osb[:], ps[:], AF.Identity, bias=bias_sb[:, ob : ob + 1]
                    )
                    nc.sync.dma_start(outT[ts(ob, P), ts(rc, RC)], osb[:])

    nc.compile()
    return nc


def _prep_inputs(x, weight, bias, n_cores=N_CORES):
    """Host-side layout marshaling (transpose / swizzle / dtype cast only)."""
    B, S, D = x.shape
    O = weight.shape[0]
    rows = B * S
    Rs = rows // n_cores
    x2 = np.ascontiguousarray(x.reshape(rows, D))
    # W^T swizzle: wts[ob, ki, kb*128+oi] = W[ob*128+oi, kb*128+ki]
    w4 = weight.reshape(O // P, P, D // P, P)  # [ob, oi, kb, ki]
    wts = np.ascontiguousarray(w4.transpose(0, 3, 2, 1)).reshape(O // P, P, D)
    wg = np.ascontiguousarray(
        weight.astype(ml_dtypes.bfloat16).reshape(P, (D * O) // P)
    )
    in_maps = []
    for c in range(n_cores):
        xT_c = np.ascontiguousarray(x2[c * Rs : (c + 1) * Rs, :].T)
        in_maps.append({"xT": xT_c, "wts": wts, "wg": wg, "biasv": bias})
    return in_maps, Rs


_program_cache = {}


def kernel(x, weight, bias, _trace=False, _trace_kwargs=None):
    os.environ.setdefault("BASS_NEVER_TRACE", "1") if not _trace else None
    x = np.asarray(x, dtype=np.float32)
    weight = np.asarray(weight, dtype=np.float32)
    bias = np.asarray(bias, dtype=np.float32)
    B, S, D = x.shape
    O = weight.shape[0]
    rows = B * S
    Rs = rows // N_CORES

    key = (Rs, D, O)
    if key not in _program_cache:
        _program_cache[key] = build_bitlinear_program(Rs, D, O)
    nc = _program_cache[key]

    in_maps, Rs = _prep_inputs(x, weight, bias)
    kw = {}
    if _trace:
        kw = dict(trace=True, trace_cores=[0], **(_trace_kwargs or {}))
    res = run_bass_kernel_spmd(nc, in_maps, list(range(N_CORES)), **kw)

    out = np.empty((rows, O), dtype=np.float32)
    for c in range(N_CORES):
        out[c * Rs : (c + 1) * Rs, :] = res.results[c]["outT"].T
    out = out.reshape(B, S, O)
    if _trace:
        return out, res
    return out


# revision 2
# speedup vs baseline: 1.1714x; 1.1714x over previous
"""BitLinear (BitNet 1.58 absmean ternary) forward on 8 trn2 NeuronCores.

Math:  gamma = mean(|W|) + 1e-8
       Wq    = clip(round(W/gamma), -1, 1)   ==  sign(w) * [|w| > gamma/2]
       out   = x @ Wq^T + bias

Sharding: data-parallel over x rows (B*S = 16384 -> 2048 rows/core),
W replicated column-stream; gamma's global |W| mean is computed from a
per-core 1/8 slice of W plus one tiny AllReduce.

Per-core device kernel:
  - gamma: DVE abs-reduce over this core's fp32 W^T slice, cross-partition
    sum via a ones-matmul on PE, 8-way AllReduce of the partial sums.
  - ternary quantization on the fly from the fp32 W^T stream:
      2*Wq = Sign(w - gamma/2) + Sign(w + gamma/2)  in {-2, 0, 2}, exact bf16
    and x is pre-scaled by 0.5 (exact in bf16) to compensate.
  - out^T[o, r] = sum_i (2Wq)^T[i,o] . (x/2)^T[i,r] : bf16 matmuls, N=512,
    fp32 PSUM accumulation, bias added during the PSUM->SBUF copy.
"""

import os
import sys

for _p in (
    "/root/.axon_site",
    "/root/.axon_site/_ro/trn_rl_repo",
    "/root/.axon_site/_ro/pypackages",
    "/opt/trn_rl_repo",
):
    if os.path.isdir(_p) and _p not in sys.path:
        sys.path.append(_p)

import numpy as np
import ml_dtypes

import concourse.bass as bass
import concourse.tile as tile
from concourse import bacc, mybir
from concourse.bass import ts
from concourse.bass_utils import run_bass_kernel_spmd

AF = mybir.ActivationFunctionType
F32 = mybir.dt.float32
BF16 = mybir.dt.bfloat16

N_CORES = 8
P = 128
RC = 512  # matmul moving free dim / psum bank


def build_bitlinear_program(R, D, O, n_cores=N_CORES):
    """Build the per-core SPMD program.

    DRAM inputs (per core):
      xbh  [D, R]           bf16   (0.5*x) shard, transposed (i, r)
      wts  [O//128, 128, D] fp32   W^T swizzled: wts[ob, ki, kb*128+oi] = W[ob*128+oi, kb*128+ki]
      wgs  [O//128//n_cores, 128, D] fp32  this core's slice of wts (gamma source)
      biasv [O]             fp32
    DRAM output:
      outT [O, R]           fp32   out^T shard (o, r)
    """
    assert R % RC == 0 and D % P == 0 and O % P == 0
    n_rc = R // RC
    n_kb = D // P
    n_ob = O // P
    assert n_ob % n_cores == 0
    n_gb = n_ob // n_cores  # gamma blocks per core
    WCH = min(1024, D)  # fp32 W chunk for quantization
    n_wch = D // WCH

    nc = bacc.Bacc(
        "TRN2",
        target_bir_lowering=False,
        debug=False,
        num_devices=n_cores,
    )
    xbh = nc.dram_tensor("xbh", [D, R], BF16, kind="ExternalInput").ap()
    wts = nc.dram_tensor("wts", [n_ob, P, D], F32, kind="ExternalInput").ap()
    wgs = nc.dram_tensor("wgs", [n_gb, P, D], F32, kind="ExternalInput").ap()
    biasv = nc.dram_tensor("biasv", [O], F32, kind="ExternalInput").ap()
    outT = nc.dram_tensor("outT", [O, R], F32, kind="ExternalOutput").ap()

    with tile.TileContext(nc) as tc:
        with (
            tc.tile_pool(name="small", bufs=1) as small,
            tc.tile_pool(name="gpool", bufs=2) as gpool,
            tc.tile_pool(name="xb", bufs=1) as xb_pool,
            tc.tile_pool(name="wf", bufs=3) as wf_pool,
            tc.tile_pool(name="sgn", bufs=2) as sgn_pool,
            tc.tile_pool(name="wq", bufs=2) as wq_pool,
            tc.tile_pool(name="osb", bufs=4) as osb_pool,
            tc.tile_pool(name="ps", bufs=4, space="PSUM") as ps_pool,
            tc.tile_pool(name="psg", bufs=1, space="PSUM") as psg_pool,
            tc.tile_pool(name="dram", bufs=1, space="DRAM") as dram_pool,
        ):
            # ---- constants / bias ----
            ones = small.tile([P, 1], F32)
            nc.vector.memset(ones[:], 1.0)
            bias_sb = small.tile([P, n_ob], F32)
            with nc.allow_non_contiguous_dma(reason="tiny one-shot bias load"):
                nc.sync.dma_start(
                    bias_sb[:], biasv.rearrange("(ob oi) -> oi ob", oi=P)
                )

            # ---- gamma partial: sum|W| over this core's slice ----
            pacc = small.tile([P, n_gb], F32)
            for gb in range(n_gb):
                g = gpool.tile([P, D], F32)
                nc.sync.dma_start(g[:], wgs[gb])
                nc.vector.tensor_reduce(
                    out=pacc[:, gb : gb + 1],
                    in_=g[:],
                    axis=mybir.AxisListType.X,
                    op=mybir.AluOpType.add,
                    apply_absolute_value=True,
                )
            pacc1 = small.tile([P, 1], F32)
            nc.vector.reduce_sum(pacc1[:], pacc[:], axis=mybir.AxisListType.X)
            ps_g = psg_pool.tile([1, 1], F32)
            nc.tensor.matmul(ps_g[:], pacc1[:], ones[:], start=True, stop=True)

            # ---- AllReduce partial sums across cores ----
            if n_cores > 1:
                sb_g = small.tile([1, P], F32)
                nc.vector.memset(sb_g[:], 0.0)
                nc.vector.tensor_copy(sb_g[0:1, 0:1], ps_g[:])
                cc_in = dram_pool.tile([1, P], F32)
                cc_out = dram_pool.tile([1, P], F32)
                nc.sync.dma_start(cc_in[:], sb_g[:])
                nc.gpsimd.collective_compute(
                    "AllReduce",
                    mybir.AluOpType.add,
                    replica_groups=[list(range(n_cores))],
                    ins=[cc_in.opt()],
                    outs=[cc_out.opt()],
                )
                sb_gr = small.tile([1, P], F32)
                nc.sync.dma_start(sb_gr[:], cc_out[:])
                g_total = sb_gr[0:1, 0:1]
            else:
                g_total = ps_g[:]

            # gamma/2 = sum/(D*O) * 0.5 + 0.5e-8
            halfg = small.tile([1, 1], F32)
            nc.vector.tensor_scalar(
                halfg[:],
                g_total,
                0.5 / float(D * O),
                0.5e-8,
                mybir.AluOpType.mult,
                mybir.AluOpType.add,
            )
            neghalfg = small.tile([1, 1], F32)
            nc.vector.tensor_scalar_mul(neghalfg[:], halfg[:], -1.0)
            halfg_b = small.tile([P, 1], F32)
            neghalfg_b = small.tile([P, 1], F32)
            nc.gpsimd.partition_broadcast(halfg_b[:], halfg[:])
            nc.gpsimd.partition_broadcast(neghalfg_b[:], neghalfg[:])

            # ---- on-the-fly ternary quantization of one W^T block ----
            def quantize_ob(ob):
                wq2 = wq_pool.tile([P, D], BF16)
                for ch in range(n_wch):
                    wf = wf_pool.tile([P, WCH], F32)
                    nc.sync.dma_start(wf[:], wts[ob, :, ts(ch, WCH)])
                    s1 = sgn_pool.tile([P, WCH], BF16, tag="s1")
                    s2 = sgn_pool.tile([P, WCH], BF16, tag="s2")
                    nc.scalar.activation(s1[:], wf[:], AF.Sign, bias=neghalfg_b[:, 0:1])
                    nc.scalar.activation(s2[:], wf[:], AF.Sign, bias=halfg_b[:, 0:1])
                    nc.vector.tensor_add(
                        out=wq2[:, ts(ch, WCH)], in0=s1[:], in1=s2[:]
                    )
                return wq2

            # quantize first block before the x loads so ACT starts early
            wq2_first = quantize_ob(0)

            # ---- x load (already bf16, pre-scaled by 0.5 on host) ----
            xbf = xb_pool.tile([P, n_kb, R], BF16)
            for kb in range(n_kb):
                nc.sync.dma_start(xbf[:, kb, :], xbh[ts(kb, P), :])

            # ---- main: out^T[ob, rc] = sum_kb wq2^T . xbf ----
            for ob in range(n_ob):
                wq2 = wq2_first if ob == 0 else quantize_ob(ob)
                for rc in range(n_rc):
                    ps = ps_pool.tile([P, RC], F32)
                    for kb in range(n_kb):
                        nc.tensor.matmul(
                            ps[:],
                            wq2[:, ts(kb, P)],
                            xbf[:, kb, ts(rc, RC)],
                            start=(kb == 0),
                            stop=(kb == n_kb - 1),
                        )
                    osb = osb_pool.tile([P, RC], F32)
                    nc.scalar.activation(
                        osb[:], ps[:], AF.Identity, bias=bias_sb[:, ob : ob + 1]
                    )
                    nc.sync.dma_start(outT[ts(ob, P), ts(rc, RC)], osb[:])

    nc.compile()
    return nc


def _prep_inputs(x, weight, bias, n_cores=N_CORES):
    """Host-side layout marshaling (transpose / swizzle / dtype cast only)."""
    B, S, D = x.shape
    O = weight.shape[0]
    rows = B * S
    Rs = rows // n_cores
    x2 = x.reshape(rows, D)
    xh = (x2 * np.float32(0.5)).astype(ml_dtypes.bfloat16)
    xbhT = np.ascontiguousarray(xh.T)  # [D, rows]
    # W^T swizzle: wts[ob, ki, kb*128+oi] = W[ob*128+oi, kb*128+ki]
    w4 = weight.reshape(O // P, P, D // P, P)  # [ob, oi, kb, ki]
    wts = np.ascontiguousarray(w4.transpose(0, 3, 2, 1)).reshape(O // P, P, D)
    n_gb = (O // P) // n_cores
    in_maps = []
    for c in range(n_cores):
        in_maps.append(
            {
                "xbh": xbhT[:, c * Rs : (c + 1) * Rs],
                "wts": wts,
                "wgs": wts[c * n_gb : (c + 1) * n_gb],
                "biasv": bias,
            }
        )
    return in_maps, Rs


_program_cache = {}


def kernel(x, weight, bias, _trace=False, _trace_kwargs=None):
    if not _trace:
        os.environ.setdefault("BASS_NEVER_TRACE", "1")
    x = np.asarray(x, dtype=np.float32)
    weight = np.asarray(weight, dtype=np.float32)
    bias = np.asarray(bias, dtype=np.float32)
    B, S, D = x.shape
    O = weight.shape[0]
    rows = B * S
    Rs = rows // N_CORES

    key = (Rs, D, O)
    if key not in _program_cache:
        _program_cache[key] = build_bitlinear_program(Rs, D, O)
    nc = _program_cache[key]

    in_maps, Rs = _prep_inputs(x, weight, bias)
    kw = {}
    if _trace:
        kw = dict(trace=True, trace_cores=[0], **(_trace_kwargs or {}))
    res = run_bass_kernel_spmd(nc, in_maps, list(range(N_CORES)), **kw)

    out = np.empty((rows, O), dtype=np.float32)
    for c in range(N_CORES):
        out[c * Rs : (c + 1) * Rs, :] = res.results[c]["outT"].T
    out = out.reshape(B, S, O)
    if _trace:
        return out, res
    return out
